# revision 41
# baseline (speedup 1.0000x reference)
"""CHESHIRE hyperedge link predictor on 8 Trainium2 NeuronCores.

Structure exploited (verified at runtime):
  - members[e] = base[e] + arange(8): each hyperedge is a contiguous
    8-node window -> sorting hyperedges by base makes the whole problem
    embarrassingly parallel across a node-range partition (no collectives).
  - duplicate windows (same base) produce identical outputs -> dedupe on
    host (~17% of windows are duplicates for the target input), compute
    unique windows only, scatter results back via the inverse map.
  - edge_index is the full directed 8-clique per hyperedge -> deg == 7,
    w == -1/7, so Lhat(h) = (h - sum(h))/7 and the K=3 ChebConv folds into
    a single per-entry GEMM plus a per-window GEMM:
        u_i = (r * ctr_i) @ Wap + (r * S) @ Wd + D0
    with r = 1/sqrt(var+eps) (GraphNorm fold), S = window sum of x,
    ctr_i = x_i - (alpha/8) S.
  - clip commutes with max/min pools; clip(u)^2 == min(u^2, 1) for the rms
    pool, so hardtanh is applied once to u before all three pools.
  - y_max - y_min never needs materializing: logits use a 3-column weight
    [w_a, -w_a, w_b] with three accumulating PE matmuls.

Per-core schedule:
  encoder GEMM (fp16 in, f32 psum) -> ACT cast+bias move to xT (bf16)
  -> DVE 4x-mode clip in place -> PE transpose -> DMA writes x rows to
  DRAM straight from the transpose PSUM -> eager SWDGE gathers (all
  chunks, gated on the exact x-write batches they need) -> per-chunk
  phase B interleaved INTO the encoder loop (stats emitted a margin of
  batches after their gather so the in-order DVE queue never stalls the
  encoder), then pipelined stats-one-chunk-ahead of gemm+pools.
  GraphNorm variance is computed CENTERED (ctr = x - alpha*mean,
  var = sum ctr^2) so bf16 arithmetic cannot cancel catastrophically.
  All 8->1 entry reductions (S, var, max, min, ssq) run as 3-op
  contiguous-half trees ([128,4,G] + [128,2,G] + [128,G]) to amortize
  the DVE per-op bubble; sigmoid is deferred to one final ACT pass.
"""

import math
import os

import numpy as np

N_CORES = 8
M = 8          # nodes per hyperedge
D = 128        # embedding dim
F = 512        # input feature dim
EPS = 1e-5
GSZ = 512      # max windows per chunk (one f32 PSUM bank)

def _stats_margin(g):
    # batches between a gather's last x-write dep and its stats emission
    # (covers SWDGE descgen + DMA latency; descgen scales with idx count)
    return 2 if g <= 256 else 3

_GRAPH_CACHE = {}
LAST_EXEC_NS = None
LAST_RESULT = None


def _bf16_dtype():
    import ml_dtypes

    return np.dtype(ml_dtypes.bfloat16)


def _fold_weights(W_enc, b_enc, gn_gamma, gn_beta, gn_alpha, cheb_W, cheb_b,
                  lin_W, lin_b):
    f32 = np.float32
    W0, W1, W2 = (np.asarray(cheb_W[i], f32) for i in range(3))
    gam = np.asarray(gn_gamma, f32)
    bet = np.asarray(gn_beta, f32)
    alp = np.asarray(gn_alpha, f32)
    Wa = W0 + W1 / f32(7.0) - f32(47.0 / 49.0) * W2
    Wb = -W1 / f32(7.0) + f32(12.0 / 49.0) * W2
    Wap = gam[:, None] * Wa
    Wd = ((f32(1.0) - alp) * gam)[:, None] * Wb
    D0 = bet @ Wa + f32(8.0) * (bet @ Wb) + np.asarray(cheb_b, f32)
    bf16 = _bf16_dtype()
    lw = np.asarray(lin_W, f32).reshape(2, 128)   # [0]=w_a (max-min), [1]=w_b
    w13 = np.stack([lw[0], -lw[0], lw[1]], axis=1)  # [128, 3]
    return {
        "wenc": np.ascontiguousarray(
            np.asarray(W_enc, f32).reshape(4, 128, 128).transpose(1, 0, 2)
            .astype(np.float16)),
        "benc": np.asarray(b_enc, f32).reshape(128, 1).copy(),
        "wap": np.ascontiguousarray(Wap.astype(bf16)),
        "wd": np.ascontiguousarray(Wd.astype(bf16)),
        "d0": np.ascontiguousarray(D0.reshape(128, 1)),
        "al8": np.ascontiguousarray((alp / f32(8.0)).reshape(128, 1)),
        "w13": np.ascontiguousarray(w13.astype(bf16)),  # [128, 3]
        "linb": np.asarray(lin_b, f32).reshape(1, 1).copy(),
        "ident": np.eye(128, dtype=bf16),
    }


def _build_graph(u_pad, chunks, gdeps, benc_zero=False):
    """Build the per-core Bass graph. SPMD: same graph on all 8 cores.

    chunks: tuple of per-chunk window counts (each %128==0, <=512).
    gdeps:  per chunk, number of 512-node x-write batches it depends on.
    """
    import concourse.bass as bass
    import concourse.tile as tile
    from concourse import bacc, mybir

    f32 = mybir.dt.float32
    bf16 = mybir.dt.bfloat16
    f16 = mybir.dt.float16
    i16 = mybir.dt.int16
    AF = mybir.ActivationFunctionType
    OP = mybir.AluOpType

    nb = u_pad // 512          # encoder column batches
    ng = len(chunks)
    e_pad = int(sum(chunks))
    offs = [0]
    for g in chunks:
        offs.append(offs[-1] + g)
    ns16 = e_pad // 16

    nc = bacc.Bacc()
    posT_p = nc.declare_dram_parameter("posT", [nb, 128, 4, 512], f16, False)
    idx_p = nc.declare_dram_parameter("idx", [128, ns16], i16, False)
    wenc_p = nc.declare_dram_parameter("wenc", [128, 4, 128], f16, False)
    benc_p = nc.declare_dram_parameter("benc", [128, 1], f32, False)
    wap_p = nc.declare_dram_parameter("wap", [128, 128], bf16, False)
    wd_p = nc.declare_dram_parameter("wd", [128, 128], bf16, False)
    d0_p = nc.declare_dram_parameter("d0", [128, 1], f32, False)
    al8_p = nc.declare_dram_parameter("al8", [128, 1], f32, False)
    w13_p = nc.declare_dram_parameter("w13", [128, 3], bf16, False)
    linb_p = nc.declare_dram_parameter("linb", [1, 1], f32, False)
    ident_p = nc.declare_dram_parameter("ident", [128, 128], bf16, False)
    out_p = nc.declare_dram_parameter("out", [1, e_pad], f32, True)

    # gather scratch: x rows, bf16, padded so overlapping window reads stay
    # in bounds
    x_dram = nc.dram_tensor("x_scratch", [u_pad + M, 128], bf16)

    # NO stats are emitted inside the encoder loop: an in-loop stats block
    # parks ACT/DVE ops (gated on its gather) AHEAD of later batches'
    # stage-copies in the in-order queues, which stalls the x-writes that
    # feed the remaining gathers — a priority inversion that paced the
    # whole back half at ~15us per gather.  With a clean encoder the
    # writes/gathers flow at full pace and the post-loop stats-ahead
    # pipeline consumes them just-in-time.
    stats_after = {}

    with tile.TileContext(nc) as tc:
        with (
            tc.tile_pool(name="consts", bufs=1) as consts,
            tc.tile_pool(name="psum_enc", bufs=2, space="PSUM") as psum_enc,
            tc.tile_pool(name="psum_tr", bufs=1, space="PSUM") as psum_tr,
            tc.tile_pool(name="psum_gemm", bufs=2, space="PSUM") as psum_gemm,
            tc.tile_pool(name="psum_log", bufs=1, space="PSUM") as psum_log,
            tc.tile_pool(name="pos", bufs=3) as pos_pool,
            tc.tile_pool(name="xt", bufs=1) as xt_pool,
            tc.tile_pool(name="stage", bufs=3) as stage_pool,
            tc.tile_pool(name="xe", bufs=3) as xe_pool,
            tc.tile_pool(name="sq", bufs=2) as sq_pool,
            tc.tile_pool(name="xs", bufs=3) as xs_pool,
            tc.tile_pool(name="cu", bufs=3) as cu_pool,
            tc.tile_pool(name="stats", bufs=4) as stats_pool,
            tc.tile_pool(name="pools", bufs=2) as pools_pool,
            tc.tile_pool(name="tsc", bufs=3) as tree_pool,
            tc.tile_pool(name="fixed", bufs=1) as fixed_pool,
        ):
            # ---- early constants (needed by encoder / in-loop stats) ----
            wenc_t = consts.tile([128, 4, 128], f16)
            nc.scalar.dma_start(out=wenc_t[:, :, :], in_=wenc_p[:, :, :])
            benc_t = consts.tile([128, 1], f32)
            nc.scalar.dma_start(out=benc_t[:, :], in_=benc_p[:, :])
            ident_t = consts.tile([128, 128], bf16)
            nc.scalar.dma_start(out=ident_t[:, :], in_=ident_p[:, :])
            idx_t = consts.tile([128, ns16], i16)
            nc.scalar.dma_start(out=idx_t[:, :], in_=idx_p[:, :])
            al8_t = consts.tile([128, 1], f32)
            nc.scalar.dma_start(out=al8_t[:, :], in_=al8_p[:, :])
            # late constants (gemm/logits, all post-loop consumers) load on
            # the sync queue after the encoder's pos DMAs are all issued
            wap_t = consts.tile([128, 128], bf16)
            wd_t = consts.tile([128, 128], bf16)
            d0_t = consts.tile([128, 1], f32)
            w13_t = consts.tile([128, 3], bf16)
            linb_t = consts.tile([1, 1], f32)
            from concourse import library_config
            nc.gpsimd.load_library(library_config.mlp)

            # pad rows of x_dram zeroed up front (HWDGE on sync; keeps the
            # gpsimd queue free for the gather descgens)
            zstg = fixed_pool.tile([M, 128], bf16, tag="zpad")
            nc.vector.memset(zstg[:, :], 0)
            padw = nc.sync.dma_start(out=x_dram[u_pad:u_pad + M, :],
                                     in_=zstg[:, :])

            xT = xt_pool.tile([128, u_pad], bf16)  # [D, node]
            sig = fixed_pool.tile([1, e_pad], f32, tag="sig")

            x_writes = []
            pending_writes = []
            xes = [None] * ng
            st = [None] * ng     # per-chunk (p_bf, xs) tiles
            stats_done = [False] * ng

            x_view = bass.AP(tensor=x_dram, offset=0,
                             ap=[[128, u_pad], [1, M * 128]])

            def vtree(dst, src, op, g):
                # 8 planes -> 1 via contiguous-half pairing, 3 ops total:
                # [128,4,g] + [128,2,g] + [128,g].  All levels bf16 step-1
                # (2x DVE mode); levels 2-3 reduce in place inside t1.
                t1 = tree_pool.tile([128, 4, GSZ], bf16, tag="tt")
                nc.vector.tensor_tensor(
                    out=t1[:, :, 0:g], in0=src[:, 0:4, 0:g],
                    in1=src[:, 4:8, 0:g], op=op)
                nc.vector.tensor_tensor(
                    out=t1[:, 0:2, 0:g], in0=t1[:, 0:2, 0:g],
                    in1=t1[:, 2:4, 0:g], op=op)
                nc.vector.tensor_tensor(out=dst, in0=t1[:, 0, 0:g],
                                        in1=t1[:, 1, 0:g], op=op)

            def emit_gather(c):
                g = chunks[c]
                xeT = xe_pool.tile([128, M, g], bf16, tag=f"xe{g}")
                xes[c] = xeT
                gi = nc.gpsimd.dma_gather(
                    out_ap=xeT[:, :, 0:g],
                    in_ap=x_view,
                    idxs_ap=idx_t[:, offs[c] // 16:offs[c + 1] // 16],
                    num_idxs=g,
                    num_idxs_reg=g,
                    elem_size=M * 128,
                    elem_step=128,
                    transpose=True,
                )
                for w in x_writes[:gdeps[c]]:
                    tile.add_dep_helper(gi.ins, w.ins, reason="x_dram RAW")
                tile.add_dep_helper(gi.ins, padw.ins, reason="x_dram pad RAW")

            def emit_stats_pre(c):
                # window stats via CENTERED variance (ctr = x - alpha*mean):
                # sum-of-squares has no cancellation, so bf16 stays accurate
                # even for near-constant windows.  The 1/8 variance scale is
                # folded into the ACT Square (scale=sqrt(1/8)), +eps into the
                # tree's last level (scalar_tensor_tensor), and the
                # reciprocal runs BEFORE the sqrt so the ACT Sqrt emits r in
                # bf16 directly (no separate f32->bf16 cast op).
                g = chunks[c]
                xeT = xes[c]
                S_bf = stats_pool.tile([128, GSZ], bf16, tag="S")
                am = stats_pool.tile([128, GSZ], bf16, tag="am")
                v = stats_pool.tile([128, GSZ], f32, tag="v")
                r_bf = stats_pool.tile([128, GSZ], bf16, tag="rb")
                p_bf = stats_pool.tile([128, GSZ], bf16, tag="p")
                sq = sq_pool.tile([128, M, GSZ], bf16, tag="sq")
                xs = xs_pool.tile([128, M, GSZ], bf16, tag="xs")

                vtree(S_bf[:, 0:g], xeT, OP.add, g)
                nc.vector.tensor_scalar(             # am = (alpha/8) * S
                    out=am[:, 0:g], in0=S_bf[:, 0:g], scalar1=al8_t[:, 0:1],
                    scalar2=None, op0=OP.mult)
                am_b = am[:, 0:g].unsqueeze(1).broadcast_to((128, M, g))
                nc.vector.tensor_tensor(out=xs[:, :, 0:g], in0=xeT[:, :, 0:g],
                                        in1=am_b, op=OP.subtract)  # ctr
                # sq = ctr^2/8, in two halves so the tree level 1 starts at
                # half the ACT latency
                nc.scalar.activation(sq[:, 0:4, 0:g], xs[:, 0:4, 0:g],
                                     AF.Square, scale=0.35355339)
                nc.scalar.activation(sq[:, 4:8, 0:g], xs[:, 4:8, 0:g],
                                     AF.Square, scale=0.35355339)
                t1 = tree_pool.tile([128, 4, GSZ], bf16, tag="tt")
                nc.vector.tensor_tensor(
                    out=t1[:, :, 0:g], in0=sq[:, 0:4, 0:g],
                    in1=sq[:, 4:8, 0:g], op=OP.add)
                nc.vector.tensor_tensor(
                    out=t1[:, 0:2, 0:g], in0=t1[:, 0:2, 0:g],
                    in1=t1[:, 2:4, 0:g], op=OP.add)
                nc.vector.scalar_tensor_tensor(      # v = var + eps
                    out=v[:, 0:g], in0=t1[:, 0, 0:g], scalar=EPS,
                    in1=t1[:, 1, 0:g], op0=OP.add, op1=OP.add)
                nc.vector.reciprocal_approx_fast(out=v[:, 0:g],
                                                 in_=v[:, 0:g])
                st[c] = (S_bf, v, r_bf, p_bf, xs)

            def emit_stats_post(c):
                # ACT sqrt + the two big DVE scale ops, emitted separately
                # so ready cu-moves / pool trees can slot in between on the
                # in-order queues
                g = chunks[c]
                S_bf, v, r_bf, p_bf, xs = st[c]
                nc.scalar.activation(r_bf[:, 0:g], v[:, 0:g], AF.Sqrt)
                nc.vector.tensor_tensor(out=p_bf[:, 0:g], in0=r_bf[:, 0:g],
                                        in1=S_bf[:, 0:g], op=OP.mult)
                # xs = ctr * r (broadcast over the 8 entry planes, in place)
                r_b = r_bf[:, 0:g].unsqueeze(1).broadcast_to((128, M, g))
                nc.vector.tensor_tensor(out=xs[:, :, 0:g], in0=xs[:, :, 0:g],
                                        in1=r_b, op=OP.mult)
                st[c] = (p_bf, xs)
                stats_done[c] = True

            def emit_stats(c):
                emit_stats_pre(c)
                emit_stats_post(c)

            cus = [None] * ng

            def emit_gemm(c):
                g = chunks[c]
                p_bf, xs = st[c]
                cu = cu_pool.tile([128, M, GSZ], bf16, tag="cu")
                cus[c] = cu
                # per-entry GEMM, 2 entries per 2-bank PSUM tile with two
                # tiles in flight: the fused ACT identity+bias move of one
                # pair overlaps the matmuls of the next
                for t in range(M // 2):
                    ps2 = psum_gemm.tile([128, 2, GSZ], f32, tag="g")
                    for j in range(2):
                        nc.tensor.matmul(ps2[:, j, 0:g], lhsT=wap_t[:, :],
                                         rhs=xs[:, 2 * t + j, 0:g],
                                         start=True, stop=False)
                    for j in range(2):
                        nc.tensor.matmul(ps2[:, j, 0:g], lhsT=wd_t[:, :],
                                         rhs=p_bf[:, 0:g],
                                         start=False, stop=True)
                    nc.scalar.activation(cu[:, 2 * t:2 * t + 2, 0:g],
                                         ps2[:, :, 0:g], AF.Identity,
                                         bias=d0_t[:, 0:1], scale=1.0)

            def emit_clip(c):
                # hardtanh once, shared by all three pools (4x DVE mode)
                g = chunks[c]
                cu = cus[c]
                nc.vector.tensor_scalar(
                    out=cu[:, :, 0:g], in0=cu[:, :, 0:g],
                    scalar1=1.0, scalar2=-1.0, op0=OP.min, op1=OP.max)

            pl = [None] * ng

            def emit_pools_a(c):
                g = chunks[c]
                cu = cus[c]
                umax = pools_pool.tile([128, GSZ], bf16, tag="ux")
                umin = pools_pool.tile([128, GSZ], bf16, tag="un")
                vtree(umax[:, 0:g], cu, OP.max, g)
                vtree(umin[:, 0:g], cu, OP.min, g)
                pl[c] = (umax, umin)

            def emit_pools_b(c):
                g = chunks[c]
                cs = slice(offs[c], offs[c + 1])
                cu = cus[c]
                umax, umin = pl[c]
                ssq = pools_pool.tile([128, GSZ], bf16, tag="ssq")
                rms = pools_pool.tile([128, GSZ], bf16, tag="rms")
                # rms pool: clip(u)^2 == min(u^2, 1); square in halves so
                # the sum tree starts at half the ACT latency
                sq2 = sq_pool.tile([128, M, GSZ], bf16, tag="sq")
                nc.scalar.activation(sq2[:, 0:4, 0:g], cu[:, 0:4, 0:g],
                                     AF.Square)
                nc.scalar.activation(sq2[:, 4:8, 0:g], cu[:, 4:8, 0:g],
                                     AF.Square)
                vtree(ssq[:, 0:g], sq2, OP.add, g)
                nc.scalar.activation(rms[:, 0:g], ssq[:, 0:g], AF.Sqrt,
                                     scale=0.125)
                # logits = umax@w_a - umin@w_a + rms@w_b  (3-col weight)
                psl = psum_log.tile([1, GSZ], f32, tag="log")
                nc.tensor.matmul(psl[:, 0:g], lhsT=w13_t[:, 0:1],
                                 rhs=umax[:, 0:g], start=True, stop=False)
                nc.tensor.matmul(psl[:, 0:g], lhsT=w13_t[:, 1:2],
                                 rhs=umin[:, 0:g], start=False, stop=False)
                nc.tensor.matmul(psl[:, 0:g], lhsT=w13_t[:, 2:3],
                                 rhs=rms[:, 0:g], start=False, stop=True)
                nc.scalar.activation(sig[:, cs], psl[:, 0:g], AF.Identity,
                                     bias=linb_t[0:1, 0:1], scale=1.0)

            # ---- encoder loop with interleaved phase-B emission ----
            for b in range(nb):
                pos_tile = pos_pool.tile([128, 4, 512], f16, tag="pos")
                nc.sync.dma_start(out=pos_tile[:, :, :], in_=posT_p[b, :, :, :])
                ps = psum_enc.tile([128, 512], f32, tag="enc")
                for k in range(4):
                    nc.tensor.matmul(
                        ps[:, :],
                        lhsT=wenc_t[:, k, :],
                        rhs=pos_tile[:, k, :],
                        start=(k == 0),
                        stop=(k == 3),
                    )
                bs = slice(b * 512, (b + 1) * 512)
                if benc_zero:
                    # single DVE op: clip + cast + PSUM->SBUF move
                    nc.vector.tensor_scalar(
                        out=xT[:, bs], in0=ps[:, :],
                        scalar1=1.0, scalar2=-1.0, op0=OP.min, op1=OP.max)
                else:
                    nc.scalar.activation(xT[:, bs], ps[:, :], AF.Identity,
                                         bias=benc_t[:, 0:1], scale=1.0)
                    nc.vector.tensor_scalar(
                        out=xT[:, bs], in0=xT[:, bs],
                        scalar1=1.0, scalar2=-1.0, op0=OP.min, op1=OP.max)
                pst = psum_tr.tile([128, 4, 128], bf16, tag="tr")
                for j in range(4):
                    t = 4 * b + j
                    nc.tensor.transpose(
                        out=pst[:, j, :],
                        in_=xT[:, t * 128:(t + 1) * 128],
                        identity=ident_t[:, :],
                    )
                stg = stage_pool.tile([128, 4, 128], bf16, tag="stage")
                nc.scalar.copy(out=stg[:, :, :], in_=pst[:, :, :])
                out_ap = bass.AP(
                    tensor=x_dram, offset=b * 512 * 128,
                    ap=[[128, 128], [128 * 128, 4], [1, 128]])
                pending_writes.append((out_ap, stg))
                # x-writes go on the sync queue with a 2-batch lag: by the
                # time the write is issued its stage copy has long landed,
                # so it never stalls the pos-load stream ahead of it
                if b >= 2:
                    wap_, wstg = pending_writes.pop(0)
                    x_writes.append(
                        nc.sync.dma_start(out=wap_, in_=wstg[:, :, :]))
                # issue any gathers whose node range is now fully written
                for c in range(ng):
                    if gdeps[c] == len(x_writes) and xes[c] is None:
                        emit_gather(c)
                for c in stats_after.get(b, []):
                    emit_stats(c)
            while pending_writes:
                wap_, wstg = pending_writes.pop(0)
                x_writes.append(
                    nc.sync.dma_start(out=wap_, in_=wstg[:, :, :]))
                for c in range(ng):
                    if gdeps[c] == len(x_writes) and xes[c] is None:
                        emit_gather(c)
            # late consts now that every pos DMA is issued on sync
            nc.sync.dma_start(out=wap_t[:, :], in_=wap_p[:, :])
            nc.sync.dma_start(out=wd_t[:, :], in_=wd_p[:, :])
            nc.sync.dma_start(out=d0_t[:, :], in_=d0_p[:, :])
            nc.sync.dma_start(out=w13_t[:, :], in_=w13_p[:, :])
            nc.sync.dma_start(out=linb_t[:, :], in_=linb_p[:, :])
            # leftover gathers (gdeps == nb), then phase B: stats one chunk
            # ahead of gemm, pools one chunk behind (so chunk c's pool/logit
            # ops never head-of-line block chunk c+1's gemm matmuls on the
            # in-order PE/DVE queues)
            for c in range(ng):
                if xes[c] is None:
                    emit_gather(c)
            if not stats_done[0]:
                emit_stats(0)
            # stats one chunk ahead of gemm, pools one chunk behind (so
            # chunk c's pool/logit ops never head-of-line block chunk c+1's
            # gemm matmuls on the in-order PE/DVE queues)
            for c in range(ng):
                if c + 1 < ng and not stats_done[c + 1]:
                    emit_stats(c + 1)
                emit_gemm(c)
                emit_clip(c)
                if c > 0:
                    emit_pools_a(c - 1)
                    emit_pools_b(c - 1)
            emit_pools_a(ng - 1)
            emit_pools_b(ng - 1)

            # final sigmoid (one act-table switch), bulk emitted before
            # the last chunk's logits land so only the last chunk sits on
            # the tail; bulk output DMA likewise overlaps the last chunk
            nc.scalar.activation(sig[:, 0:offs[ng - 1]],
                                 sig[:, 0:offs[ng - 1]], AF.Sigmoid)
            nc.sync.dma_start(out=out_p[:, 0:offs[ng - 1]],
                              in_=sig[:, 0:offs[ng - 1]])
            nc.scalar.activation(sig[:, offs[ng - 1]:],
                                 sig[:, offs[ng - 1]:], AF.Sigmoid)
            nc.sync.dma_start(out=out_p[:, offs[ng - 1]:],
                              in_=sig[:, offs[ng - 1]:])

    nc.finalize()
    return nc


def _np_fallback(pos_set, W_enc, b_enc, gn_gamma, gn_beta, gn_alpha, cheb_W,
                 cheb_b, lin_W, lin_b, members, edge_index, batch):
    """Pure-numpy general path (only used if the expected input structure is
    absent; inputs from setup_inputs always take the device path)."""
    f32 = np.float32
    E = members.shape[0]
    num_entries = members.size
    x = np.clip(pos_set @ W_enc + b_enc, -1.0, 1.0).astype(f32)
    xe = x[members.reshape(-1)]
    cnt = np.zeros(E, f32)
    np.add.at(cnt, batch, 1.0)
    mean = np.zeros((E, x.shape[1]), f32)
    np.add.at(mean, batch, xe)
    mean /= cnt[:, None]
    ctr = xe - gn_alpha * mean[batch]
    var = np.zeros((E, x.shape[1]), f32)
    np.add.at(var, batch, ctr * ctr)
    var /= cnt[:, None]
    xe = gn_gamma * ctr / np.sqrt(var + EPS)[batch] + gn_beta
    src, dst = edge_index[0], edge_index[1]
    deg = np.zeros(num_entries, f32)
    np.add.at(deg, dst, 1.0)
    w = -1.0 / np.sqrt(deg[src] * deg[dst])

    def lhat(h):
        out = np.zeros_like(h)
        np.add.at(out, dst, w[:, None] * h[src])
        return out

    tx0 = xe
    tx1 = lhat(tx0)
    out = tx0 @ cheb_W[0] + tx1 @ cheb_W[1]
    tkm1, tkm2 = tx1, tx0
    for k in range(2, cheb_W.shape[0]):
        tk = 2.0 * lhat(tkm1) - tkm2
        out = out + tk @ cheb_W[k]
        tkm1, tkm2 = tk, tkm1
    h = np.clip(out + cheb_b, -1.0, 1.0)
    ymax = np.full((E, h.shape[1]), -np.inf, f32)
    ymin = np.full((E, h.shape[1]), np.inf, f32)
    np.maximum.at(ymax, batch, h)
    np.minimum.at(ymin, batch, h)
    ynorm = np.zeros((E, h.shape[1]), f32)
    np.add.at(ynorm, batch, h * h)
    ynorm = np.sqrt(ynorm / cnt[:, None])
    y = np.concatenate([ymax - ymin, ynorm], axis=1)
    logits = y @ lin_W + lin_b
    return (1.0 / (1.0 + np.exp(-logits))).squeeze(-1).astype(f32)


def _has_window_structure(members, edge_index, batch):
    E, Mm = members.shape
    if Mm != M:
        return False
    base = members[:, 0]
    if not (members == base[:, None] + np.arange(M, dtype=members.dtype)).all():
        return False
    if not (batch == np.repeat(np.arange(E, dtype=batch.dtype), M)).all():
        return False
    row, col = np.where(~np.eye(M, dtype=bool))
    offs = np.arange(E, dtype=np.int64)[:, None] * M
    ei = np.stack([(offs + row[None, :]).ravel(), (offs + col[None, :]).ravel()])
    return (edge_index == ei).all()


def kernel(pos_set, W_enc, b_enc, gn_gamma, gn_beta, gn_alpha, cheb_W, cheb_b,
           lin_W, lin_b, members, edge_index, batch):
    pos_set = np.asarray(pos_set, np.float32)
    members = np.asarray(members)
    edge_index = np.asarray(edge_index)
    batch = np.asarray(batch)
    if not _has_window_structure(members, edge_index, batch):
        return _np_fallback(
            pos_set, np.asarray(W_enc, np.float32),
            np.asarray(b_enc, np.float32), np.asarray(gn_gamma, np.float32),
            np.asarray(gn_beta, np.float32), np.asarray(gn_alpha, np.float32),
            np.asarray(cheb_W, np.float32), np.asarray(cheb_b, np.float32),
            np.asarray(lin_W, np.float32), np.asarray(lin_b, np.float32),
            members, edge_index, batch)

    N = pos_set.shape[0]
    base_all = members[:, 0].astype(np.int64)
    # duplicate windows (same base) give identical outputs: compute unique
    # windows only, scatter back through the inverse map at the end
    uniq, inv = np.unique(base_all, return_inverse=True)
    EU = uniq.size
    base = uniq                                              # sorted
    node_span = (N + N_CORES - 1) // N_CORES
    u_pad = ((node_span + M + 511) // 512 + 1) * 512
    # quantile split: equal window count per core (bases already sorted)
    ecnt = (EU + N_CORES - 1) // N_CORES
    counts = np.array([max(0, min(ecnt, EU - c * ecnt)) for c in range(N_CORES)])
    offs_pre = np.concatenate([[0], np.cumsum(counts)])
    node_lo = np.zeros(N_CORES, np.int64)
    ok = True
    for c in range(N_CORES):
        lo_i, hi_i = offs_pre[c], offs_pre[c + 1]
        if hi_i == lo_i:
            node_lo[c] = 0
            continue
        node_lo[c] = base[lo_i]
        if base[hi_i - 1] + M - node_lo[c] > u_pad:
            ok = False
            break
    if not ok:
        core_of = np.minimum(base // node_span, N_CORES - 1)
        counts = np.bincount(core_of, minlength=N_CORES)
        offs_pre = np.concatenate([[0], np.cumsum(counts)])
        node_lo = np.arange(N_CORES, dtype=np.int64) * node_span
    cmax = int(counts.max())
    # lead with small chunks so the first gathers (and their stats) are
    # ready within a few encoder batches and fill the otherwise-idle DVE
    chunks = []
    left = cmax
    while left > 0:
        take = min(GSZ, ((left + 127) // 128) * 128)
        chunks.append(take)
        left -= take
    chunks = tuple(chunks)
    e_pad = int(sum(chunks))
    coffs = np.concatenate([[0], np.cumsum(chunks)]).astype(np.int64)

    nb_ = u_pad // 512
    # per-chunk: how many 512-node x-write batches the gather depends on
    # (max over cores, from the actual window bases)
    gdeps = []
    for c in range(len(chunks)):
        mx = 0
        for cc in range(N_CORES):
            lo_i = offs_pre[cc] + coffs[c]
            hi_i = min(offs_pre[cc] + coffs[c + 1], offs_pre[cc + 1])
            if hi_i > lo_i:
                mx = max(mx, int((base[lo_i:hi_i] - node_lo[cc]).max()))
        gdeps.append(min(nb_, max(1, (mx + M + 511) // 512)))
    gdeps = tuple(gdeps)
    benc_zero = bool(np.all(np.asarray(b_enc) == 0.0))
    key = (u_pad, chunks, gdeps, benc_zero)
    if key not in _GRAPH_CACHE:
        _GRAPH_CACHE[key] = _build_graph(u_pad, chunks, gdeps, benc_zero)
    nc = _GRAPH_CACHE[key]

    shared = _fold_weights(W_enc, b_enc, gn_gamma, gn_beta, gn_alpha, cheb_W,
                           cheb_b, lin_W, lin_b)
    nb = u_pad // 512
    ns16 = e_pad // 16

    in_maps = []
    for c in range(N_CORES):
        lo = int(node_lo[c])
        sl = pos_set[lo:min(lo + u_pad, N)]
        if sl.shape[0] < u_pad:
            sl = np.concatenate(
                [sl, np.zeros((u_pad - sl.shape[0], F), np.float32)], 0)
        # posT[b, p, k, u'] = sl[512b+u', 128k+p]
        posT = np.ascontiguousarray(
            sl.reshape(nb, 512, 4, 128).transpose(0, 3, 2, 1)
            .astype(np.float16))
        wins = base[offs_pre[c]:offs_pre[c + 1]]
        loc = (wins - lo).astype(np.int64)
        idx = np.zeros(e_pad, np.int16)
        # place each chunk's windows at its padded chunk offset
        for ci in range(len(chunks)):
            seg = loc[coffs[ci]:min(coffs[ci + 1], loc.size)]
            idx[int(coffs[ci]):int(coffs[ci]) + seg.size] = \
                seg.astype(np.int16)
        # wrapped layout: element i lives at [i % 16, i // 16], replicated
        # across the eight 16-partition groups
        w16 = idx.reshape(ns16, 16).T           # [16, ns16]
        m = dict(shared)
        m["posT"] = posT
        m["idx"] = np.ascontiguousarray(np.tile(w16, (8, 1)))
        in_maps.append(m)

    from concourse.bass_utils import run_bass_kernel_spmd

    trace = bool(os.environ.get("CHESHIRE_TRACE"))
    res = run_bass_kernel_spmd(nc, in_maps, core_ids=list(range(N_CORES)),
                               trace=trace)
    global LAST_EXEC_NS, LAST_RESULT
    LAST_EXEC_NS = res.exec_time_ns
    LAST_RESULT = res
    out_u = np.zeros(EU, np.float32)
    for c in range(N_CORES):
        cnt_c = int(counts[c])
        if cnt_c == 0:
            continue
        vals = np.asarray(res.results[c]["out"], np.float32).reshape(-1)
        # undo chunk padding: chunk ci's real windows sit at coffs[ci]
        got = np.empty(cnt_c, np.float32)
        for ci in range(len(chunks)):
            s = int(coffs[ci])
            e = min(int(coffs[ci + 1]), cnt_c)
            if e > s:
                got[s:e] = vals[s:e]
        out_u[offs_pre[c]:offs_pre[c] + cnt_c] = got
    return out_u[inv]


# revision 43
# speedup vs baseline: 1.0404x; 1.0404x over previous
"""CHESHIRE hyperedge link predictor on 8 Trainium2 NeuronCores.

Structure exploited (verified at runtime):
  - members[e] = base[e] + arange(8): each hyperedge is a contiguous
    8-node window -> sorting hyperedges by base makes the whole problem
    embarrassingly parallel across a node-range partition (no collectives).
  - duplicate windows (same base) produce identical outputs -> dedupe on
    host (~17% of windows are duplicates for the target input), compute
    unique windows only, scatter results back via the inverse map.
  - edge_index is the full directed 8-clique per hyperedge -> deg == 7,
    w == -1/7, so Lhat(h) = (h - sum(h))/7 and the K=3 ChebConv folds into
    a single per-entry GEMM plus a per-window GEMM:
        u_i = (r * ctr_i) @ Wap + (r * S) @ Wd + D0
    with r = 1/sqrt(var+eps) (GraphNorm fold), S = window sum of x,
    ctr_i = x_i - (alpha/8) S.
  - clip commutes with max/min pools; clip(u)^2 == min(u^2, 1) for the rms
    pool, so hardtanh is applied once to u before all three pools.
  - y_max - y_min never needs materializing: logits use a 3-column weight
    [w_a, -w_a, w_b] with three accumulating PE matmuls.

Per-core schedule:
  encoder GEMM (fp16 in, f32 psum) -> ACT cast+bias move to xT (bf16)
  -> DVE 4x-mode clip in place -> PE transpose -> DMA writes x rows to
  DRAM straight from the transpose PSUM -> eager SWDGE gathers (all
  chunks, gated on the exact x-write batches they need) -> per-chunk
  phase B interleaved INTO the encoder loop (stats emitted a margin of
  batches after their gather so the in-order DVE queue never stalls the
  encoder), then pipelined stats-one-chunk-ahead of gemm+pools.
  GraphNorm variance is computed CENTERED (ctr = x - alpha*mean,
  var = sum ctr^2) so bf16 arithmetic cannot cancel catastrophically.
  All 8->1 entry reductions (S, var, max, min, ssq) run as 3-op
  contiguous-half trees ([128,4,G] + [128,2,G] + [128,G]) to amortize
  the DVE per-op bubble; sigmoid is deferred to one final ACT pass.
"""

import math
import os

import numpy as np

N_CORES = 8
M = 8          # nodes per hyperedge
D = 128        # embedding dim
F = 512        # input feature dim
EPS = 1e-5
GSZ = 512      # max windows per chunk (one f32 PSUM bank)

def _stats_margin(g):
    # batches between a gather's last x-write dep and its stats emission
    # (covers SWDGE descgen + DMA latency; descgen scales with idx count)
    return 2 if g <= 256 else 3

_GRAPH_CACHE = {}
LAST_EXEC_NS = None
LAST_RESULT = None


def _bf16_dtype():
    import ml_dtypes

    return np.dtype(ml_dtypes.bfloat16)


def _fold_weights(W_enc, b_enc, gn_gamma, gn_beta, gn_alpha, cheb_W, cheb_b,
                  lin_W, lin_b):
    f32 = np.float32
    W0, W1, W2 = (np.asarray(cheb_W[i], f32) for i in range(3))
    gam = np.asarray(gn_gamma, f32)
    bet = np.asarray(gn_beta, f32)
    alp = np.asarray(gn_alpha, f32)
    Wa = W0 + W1 / f32(7.0) - f32(47.0 / 49.0) * W2
    Wb = -W1 / f32(7.0) + f32(12.0 / 49.0) * W2
    Wap = gam[:, None] * Wa
    Wd = ((f32(1.0) - alp) * gam)[:, None] * Wb
    D0 = bet @ Wa + f32(8.0) * (bet @ Wb) + np.asarray(cheb_b, f32)
    bf16 = _bf16_dtype()
    lw = np.asarray(lin_W, f32).reshape(2, 128)   # [0]=w_a (max-min), [1]=w_b
    w13 = np.stack([lw[0], -lw[0], lw[1]], axis=1)  # [128, 3]
    return {
        "wenc": np.ascontiguousarray(
            np.asarray(W_enc, f32).reshape(4, 128, 128).transpose(1, 0, 2)
            .astype(np.float16)),
        "benc": np.asarray(b_enc, f32).reshape(128, 1).copy(),
        "wap": np.ascontiguousarray(Wap.astype(bf16)),
        "wd": np.ascontiguousarray(Wd.astype(bf16)),
        "d0": np.ascontiguousarray(D0.reshape(128, 1)),
        "al8": np.ascontiguousarray((alp / f32(8.0)).reshape(128, 1)),
        "w13": np.ascontiguousarray(w13.astype(bf16)),  # [128, 3]
        "linb": np.asarray(lin_b, f32).reshape(1, 1).copy(),
        "ident": np.eye(128, dtype=bf16),
    }


def _build_graph(u_pad, chunks, gdeps, benc_zero=False):
    """Build the per-core Bass graph. SPMD: same graph on all 8 cores.

    chunks: tuple of per-chunk window counts (each %128==0, <=512).
    gdeps:  per chunk, number of 512-node x-write batches it depends on.
    """
    import concourse.bass as bass
    import concourse.tile as tile
    from concourse import bacc, mybir

    f32 = mybir.dt.float32
    bf16 = mybir.dt.bfloat16
    f16 = mybir.dt.float16
    i16 = mybir.dt.int16
    AF = mybir.ActivationFunctionType
    OP = mybir.AluOpType

    nb = u_pad // 512          # encoder column batches
    ng = len(chunks)
    e_pad = int(sum(chunks))
    offs = [0]
    for g in chunks:
        offs.append(offs[-1] + g)
    ns16 = e_pad // 16

    nc = bacc.Bacc()
    posT_p = nc.declare_dram_parameter("posT", [nb, 128, 4, 512], f16, False)
    idx_p = nc.declare_dram_parameter("idx", [128, ns16], i16, False)
    wenc_p = nc.declare_dram_parameter("wenc", [128, 4, 128], f16, False)
    benc_p = nc.declare_dram_parameter("benc", [128, 1], f32, False)
    wap_p = nc.declare_dram_parameter("wap", [128, 128], bf16, False)
    wd_p = nc.declare_dram_parameter("wd", [128, 128], bf16, False)
    d0_p = nc.declare_dram_parameter("d0", [128, 1], f32, False)
    al8_p = nc.declare_dram_parameter("al8", [128, 1], f32, False)
    w13_p = nc.declare_dram_parameter("w13", [128, 3], bf16, False)
    linb_p = nc.declare_dram_parameter("linb", [1, 1], f32, False)
    ident_p = nc.declare_dram_parameter("ident", [128, 128], bf16, False)
    out_p = nc.declare_dram_parameter("out", [1, e_pad], f32, True)

    # gather scratch: x rows, bf16, padded so overlapping window reads stay
    # in bounds
    x_dram = nc.dram_tensor("x_scratch", [u_pad + M, 128], bf16)

    # NO stats are emitted inside the encoder loop: an in-loop stats block
    # parks ACT/DVE ops (gated on its gather) AHEAD of later batches'
    # stage-copies in the in-order queues, which stalls the x-writes that
    # feed the remaining gathers — a priority inversion that paced the
    # whole back half at ~15us per gather.  With a clean encoder the
    # writes/gathers flow at full pace and the post-loop stats-ahead
    # pipeline consumes them just-in-time.
    stats_after = {}

    with tile.TileContext(nc) as tc:
        with (
            tc.tile_pool(name="consts", bufs=1) as consts,
            tc.tile_pool(name="psum_enc", bufs=2, space="PSUM") as psum_enc,
            tc.tile_pool(name="psum_tr", bufs=3, space="PSUM") as psum_tr,
            tc.tile_pool(name="psum_gemm", bufs=1, space="PSUM") as psum_gemm,
            tc.tile_pool(name="psum_log", bufs=1, space="PSUM") as psum_log,
            tc.tile_pool(name="pos", bufs=3) as pos_pool,
            tc.tile_pool(name="xt", bufs=1) as xt_pool,
            tc.tile_pool(name="stage", bufs=5) as stage_pool,
            tc.tile_pool(name="xe", bufs=3) as xe_pool,
            tc.tile_pool(name="sq", bufs=2) as sq_pool,
            tc.tile_pool(name="xs", bufs=3) as xs_pool,
            tc.tile_pool(name="cu", bufs=3) as cu_pool,
            tc.tile_pool(name="stats", bufs=4) as stats_pool,
            tc.tile_pool(name="pools", bufs=2) as pools_pool,
            tc.tile_pool(name="tsc", bufs=3) as tree_pool,
            tc.tile_pool(name="fixed", bufs=1) as fixed_pool,
        ):
            # ---- early constants (needed by encoder / in-loop stats) ----
            wenc_t = consts.tile([128, 4, 128], f16)
            nc.scalar.dma_start(out=wenc_t[:, :, :], in_=wenc_p[:, :, :])
            benc_t = consts.tile([128, 1], f32)
            nc.scalar.dma_start(out=benc_t[:, :], in_=benc_p[:, :])
            ident_t = consts.tile([128, 128], bf16)
            nc.scalar.dma_start(out=ident_t[:, :], in_=ident_p[:, :])
            idx_t = consts.tile([128, ns16], i16)
            nc.scalar.dma_start(out=idx_t[:, :], in_=idx_p[:, :])
            al8_t = consts.tile([128, 1], f32)
            nc.scalar.dma_start(out=al8_t[:, :], in_=al8_p[:, :])
            # late constants (gemm/logits, all post-loop consumers) load on
            # the sync queue after the encoder's pos DMAs are all issued
            wap_t = consts.tile([128, 128], bf16)
            wd_t = consts.tile([128, 128], bf16)
            d0_t = consts.tile([128, 1], f32)
            w13_t = consts.tile([128, 3], bf16)
            linb_t = consts.tile([1, 1], f32)
            from concourse import library_config
            nc.gpsimd.load_library(library_config.mlp)

            # pad rows of x_dram zeroed up front (HWDGE on sync; keeps the
            # gpsimd queue free for the gather descgens)
            zstg = fixed_pool.tile([M, 128], bf16, tag="zpad")
            nc.vector.memset(zstg[:, :], 0)
            padw = nc.sync.dma_start(out=x_dram[u_pad:u_pad + M, :],
                                     in_=zstg[:, :])

            xT = xt_pool.tile([128, u_pad], bf16)  # [D, node]
            sig = fixed_pool.tile([1, e_pad], f32, tag="sig")

            x_writes = []
            pending_writes = []
            xes = [None] * ng
            st = [None] * ng     # per-chunk (p_bf, xs) tiles
            stats_done = [False] * ng

            x_view = bass.AP(tensor=x_dram, offset=0,
                             ap=[[128, u_pad], [1, M * 128]])

            def vtree(dst, src, op, g):
                # 8 planes -> 1 via contiguous-half pairing, 3 ops total:
                # [128,4,g] + [128,2,g] + [128,g].  All levels bf16 step-1
                # (2x DVE mode); levels 2-3 reduce in place inside t1.
                t1 = tree_pool.tile([128, 4, GSZ], bf16, tag="tt")
                nc.vector.tensor_tensor(
                    out=t1[:, :, 0:g], in0=src[:, 0:4, 0:g],
                    in1=src[:, 4:8, 0:g], op=op)
                nc.vector.tensor_tensor(
                    out=t1[:, 0:2, 0:g], in0=t1[:, 0:2, 0:g],
                    in1=t1[:, 2:4, 0:g], op=op)
                nc.vector.tensor_tensor(out=dst, in0=t1[:, 0, 0:g],
                                        in1=t1[:, 1, 0:g], op=op)

            def emit_gather(c):
                g = chunks[c]
                xeT = xe_pool.tile([128, M, g], bf16, tag=f"xe{g}")
                xes[c] = xeT
                gi = nc.gpsimd.dma_gather(
                    out_ap=xeT[:, :, 0:g],
                    in_ap=x_view,
                    idxs_ap=idx_t[:, offs[c] // 16:offs[c + 1] // 16],
                    num_idxs=g,
                    num_idxs_reg=g,
                    elem_size=M * 128,
                    elem_step=128,
                    transpose=True,
                )
                for w in x_writes[:gdeps[c]]:
                    tile.add_dep_helper(gi.ins, w.ins, reason="x_dram RAW")
                tile.add_dep_helper(gi.ins, padw.ins, reason="x_dram pad RAW")

            def emit_stats_pre(c):
                # window stats via CENTERED variance (ctr = x - alpha*mean):
                # sum-of-squares has no cancellation, so bf16 stays accurate
                # even for near-constant windows.  The 1/8 variance scale is
                # folded into the ACT Square (scale=sqrt(1/8)), +eps into the
                # tree's last level (scalar_tensor_tensor), and the
                # reciprocal runs BEFORE the sqrt so the ACT Sqrt emits r in
                # bf16 directly (no separate f32->bf16 cast op).
                g = chunks[c]
                xeT = xes[c]
                S_bf = stats_pool.tile([128, GSZ], bf16, tag="S")
                am = stats_pool.tile([128, GSZ], bf16, tag="am")
                v = stats_pool.tile([128, GSZ], f32, tag="v")
                r_bf = stats_pool.tile([128, GSZ], bf16, tag="rb")
                p_bf = stats_pool.tile([128, GSZ], bf16, tag="p")
                sq = sq_pool.tile([128, M, GSZ], bf16, tag="sq")
                xs = xs_pool.tile([128, M, GSZ], bf16, tag="xs")

                vtree(S_bf[:, 0:g], xeT, OP.add, g)
                nc.vector.tensor_scalar(             # am = (alpha/8) * S
                    out=am[:, 0:g], in0=S_bf[:, 0:g], scalar1=al8_t[:, 0:1],
                    scalar2=None, op0=OP.mult)
                am_b = am[:, 0:g].unsqueeze(1).broadcast_to((128, M, g))
                nc.vector.tensor_tensor(out=xs[:, :, 0:g], in0=xeT[:, :, 0:g],
                                        in1=am_b, op=OP.subtract)  # ctr
                # sq = ctr^2/8, in two halves so the tree level 1 starts at
                # half the ACT latency
                nc.scalar.activation(sq[:, 0:4, 0:g], xs[:, 0:4, 0:g],
                                     AF.Square, scale=0.35355339)
                nc.scalar.activation(sq[:, 4:8, 0:g], xs[:, 4:8, 0:g],
                                     AF.Square, scale=0.35355339)
                t1 = tree_pool.tile([128, 4, GSZ], bf16, tag="tt")
                nc.vector.tensor_tensor(
                    out=t1[:, :, 0:g], in0=sq[:, 0:4, 0:g],
                    in1=sq[:, 4:8, 0:g], op=OP.add)
                nc.vector.tensor_tensor(
                    out=t1[:, 0:2, 0:g], in0=t1[:, 0:2, 0:g],
                    in1=t1[:, 2:4, 0:g], op=OP.add)
                nc.vector.scalar_tensor_tensor(      # v = var + eps
                    out=v[:, 0:g], in0=t1[:, 0, 0:g], scalar=EPS,
                    in1=t1[:, 1, 0:g], op0=OP.add, op1=OP.add)
                nc.vector.reciprocal_approx_fast(out=v[:, 0:g],
                                                 in_=v[:, 0:g])
                st[c] = (S_bf, v, r_bf, p_bf, xs)

            def emit_stats_post(c):
                # ACT sqrt + the two big DVE scale ops, emitted separately
                # so ready cu-moves / pool trees can slot in between on the
                # in-order queues
                g = chunks[c]
                S_bf, v, r_bf, p_bf, xs = st[c]
                nc.scalar.activation(r_bf[:, 0:g], v[:, 0:g], AF.Sqrt)
                nc.vector.tensor_tensor(out=p_bf[:, 0:g], in0=r_bf[:, 0:g],
                                        in1=S_bf[:, 0:g], op=OP.mult)
                # xs = ctr * r (broadcast over the 8 entry planes, in place)
                r_b = r_bf[:, 0:g].unsqueeze(1).broadcast_to((128, M, g))
                nc.vector.tensor_tensor(out=xs[:, :, 0:g], in0=xs[:, :, 0:g],
                                        in1=r_b, op=OP.mult)
                st[c] = (p_bf, xs)
                stats_done[c] = True

            def emit_stats(c):
                emit_stats_pre(c)
                emit_stats_post(c)

            cus = [None] * ng

            def emit_gemm(c):
                g = chunks[c]
                p_bf, xs = st[c]
                cu = cu_pool.tile([128, M, GSZ], bf16, tag="cu")
                cus[c] = cu
                # per-entry GEMM, 2 entries per 2-bank PSUM tile with two
                # tiles in flight: the fused ACT identity+bias move of one
                # pair overlaps the matmuls of the next
                for t in range(M // 2):
                    ps2 = psum_gemm.tile([128, 2, GSZ], f32, tag="g")
                    for j in range(2):
                        nc.tensor.matmul(ps2[:, j, 0:g], lhsT=wap_t[:, :],
                                         rhs=xs[:, 2 * t + j, 0:g],
                                         start=True, stop=False)
                    for j in range(2):
                        nc.tensor.matmul(ps2[:, j, 0:g], lhsT=wd_t[:, :],
                                         rhs=p_bf[:, 0:g],
                                         start=False, stop=True)
                    nc.scalar.activation(cu[:, 2 * t:2 * t + 2, 0:g],
                                         ps2[:, :, 0:g], AF.Identity,
                                         bias=d0_t[:, 0:1], scale=1.0)

            def emit_clip(c):
                # hardtanh once, shared by all three pools (4x DVE mode)
                g = chunks[c]
                cu = cus[c]
                nc.vector.tensor_scalar(
                    out=cu[:, :, 0:g], in0=cu[:, :, 0:g],
                    scalar1=1.0, scalar2=-1.0, op0=OP.min, op1=OP.max)

            pl = [None] * ng

            def emit_pools_a(c):
                g = chunks[c]
                cu = cus[c]
                umax = pools_pool.tile([128, GSZ], bf16, tag="ux")
                umin = pools_pool.tile([128, GSZ], bf16, tag="un")
                vtree(umax[:, 0:g], cu, OP.max, g)
                vtree(umin[:, 0:g], cu, OP.min, g)
                pl[c] = (umax, umin)

            def emit_pools_b(c):
                g = chunks[c]
                cs = slice(offs[c], offs[c + 1])
                cu = cus[c]
                umax, umin = pl[c]
                ssq = pools_pool.tile([128, GSZ], bf16, tag="ssq")
                rms = pools_pool.tile([128, GSZ], bf16, tag="rms")
                # rms pool: clip(u)^2 == min(u^2, 1); square in halves so
                # the sum tree starts at half the ACT latency
                sq2 = sq_pool.tile([128, M, GSZ], bf16, tag="sq")
                nc.scalar.activation(sq2[:, 0:4, 0:g], cu[:, 0:4, 0:g],
                                     AF.Square)
                nc.scalar.activation(sq2[:, 4:8, 0:g], cu[:, 4:8, 0:g],
                                     AF.Square)
                vtree(ssq[:, 0:g], sq2, OP.add, g)
                nc.scalar.activation(rms[:, 0:g], ssq[:, 0:g], AF.Sqrt,
                                     scale=0.125)
                # logits = umax@w_a - umin@w_a + rms@w_b  (3-col weight)
                psl = psum_log.tile([1, GSZ], f32, tag="log")
                nc.tensor.matmul(psl[:, 0:g], lhsT=w13_t[:, 0:1],
                                 rhs=umax[:, 0:g], start=True, stop=False)
                nc.tensor.matmul(psl[:, 0:g], lhsT=w13_t[:, 1:2],
                                 rhs=umin[:, 0:g], start=False, stop=False)
                nc.tensor.matmul(psl[:, 0:g], lhsT=w13_t[:, 2:3],
                                 rhs=rms[:, 0:g], start=False, stop=True)
                nc.scalar.activation(sig[:, cs], psl[:, 0:g], AF.Identity,
                                     bias=linb_t[0:1, 0:1], scale=1.0)

            # ---- encoder loop with interleaved phase-B emission ----
            for b in range(nb):
                pos_tile = pos_pool.tile([128, 4, 512], f16, tag="pos")
                nc.sync.dma_start(out=pos_tile[:, :, :], in_=posT_p[b, :, :, :])
                ps = psum_enc.tile([128, 512], f32, tag="enc")
                for k in range(4):
                    nc.tensor.matmul(
                        ps[:, :],
                        lhsT=wenc_t[:, k, :],
                        rhs=pos_tile[:, k, :],
                        start=(k == 0),
                        stop=(k == 3),
                    )
                bs = slice(b * 512, (b + 1) * 512)
                if benc_zero:
                    # single DVE op: clip + cast + PSUM->SBUF move
                    nc.vector.tensor_scalar(
                        out=xT[:, bs], in0=ps[:, :],
                        scalar1=1.0, scalar2=-1.0, op0=OP.min, op1=OP.max)
                else:
                    nc.scalar.activation(xT[:, bs], ps[:, :], AF.Identity,
                                         bias=benc_t[:, 0:1], scale=1.0)
                    nc.vector.tensor_scalar(
                        out=xT[:, bs], in0=xT[:, bs],
                        scalar1=1.0, scalar2=-1.0, op0=OP.min, op1=OP.max)
                pst = psum_tr.tile([128, 4, 128], bf16, tag="tr")
                for j in range(4):
                    t = 4 * b + j
                    nc.tensor.transpose(
                        out=pst[:, j, :],
                        in_=xT[:, t * 128:(t + 1) * 128],
                        identity=ident_t[:, :],
                    )
                stg = stage_pool.tile([128, 4, 128], bf16, tag="stage")
                nc.scalar.copy(out=stg[:, :, :], in_=pst[:, :, :])
                out_ap = bass.AP(
                    tensor=x_dram, offset=b * 512 * 128,
                    ap=[[128, 128], [128 * 128, 4], [1, 128]])
                pending_writes.append((out_ap, stg))
                # x-writes go on the sync queue with a 2-batch lag: by the
                # time the write is issued its stage copy has long landed,
                # so it never stalls the pos-load stream ahead of it
                if b >= 2:
                    wap_, wstg = pending_writes.pop(0)
                    x_writes.append(
                        nc.sync.dma_start(out=wap_, in_=wstg[:, :, :]))
                # issue any gathers whose node range is now fully written
                for c in range(ng):
                    if gdeps[c] == len(x_writes) and xes[c] is None:
                        emit_gather(c)
                for c in stats_after.get(b, []):
                    emit_stats(c)
            while pending_writes:
                wap_, wstg = pending_writes.pop(0)
                x_writes.append(
                    nc.sync.dma_start(out=wap_, in_=wstg[:, :, :]))
                for c in range(ng):
                    if gdeps[c] == len(x_writes) and xes[c] is None:
                        emit_gather(c)
            # late consts now that every pos DMA is issued on sync
            nc.sync.dma_start(out=wap_t[:, :], in_=wap_p[:, :])
            nc.sync.dma_start(out=wd_t[:, :], in_=wd_p[:, :])
            nc.sync.dma_start(out=d0_t[:, :], in_=d0_p[:, :])
            nc.sync.dma_start(out=w13_t[:, :], in_=w13_p[:, :])
            nc.sync.dma_start(out=linb_t[:, :], in_=linb_p[:, :])
            # leftover gathers (gdeps == nb), then phase B: stats one chunk
            # ahead of gemm, pools one chunk behind (so chunk c's pool/logit
            # ops never head-of-line block chunk c+1's gemm matmuls on the
            # in-order PE/DVE queues)
            for c in range(ng):
                if xes[c] is None:
                    emit_gather(c)
            if not stats_done[0]:
                emit_stats(0)
            # stats one chunk ahead of gemm, pools one chunk behind (so
            # chunk c's pool/logit ops never head-of-line block chunk c+1's
            # gemm matmuls on the in-order PE/DVE queues)
            for c in range(ng):
                if c + 1 < ng and not stats_done[c + 1]:
                    emit_stats(c + 1)
                emit_gemm(c)
                emit_clip(c)
                if c > 0:
                    emit_pools_a(c - 1)
                    emit_pools_b(c - 1)
            emit_pools_a(ng - 1)
            emit_pools_b(ng - 1)

            # final sigmoid (one act-table switch), bulk emitted before
            # the last chunk's logits land so only the last chunk sits on
            # the tail; bulk output DMA likewise overlaps the last chunk
            nc.scalar.activation(sig[:, 0:offs[ng - 1]],
                                 sig[:, 0:offs[ng - 1]], AF.Sigmoid)
            nc.sync.dma_start(out=out_p[:, 0:offs[ng - 1]],
                              in_=sig[:, 0:offs[ng - 1]])
            nc.scalar.activation(sig[:, offs[ng - 1]:],
                                 sig[:, offs[ng - 1]:], AF.Sigmoid)
            nc.sync.dma_start(out=out_p[:, offs[ng - 1]:],
                              in_=sig[:, offs[ng - 1]:])

    nc.finalize()
    return nc


def _np_fallback(pos_set, W_enc, b_enc, gn_gamma, gn_beta, gn_alpha, cheb_W,
                 cheb_b, lin_W, lin_b, members, edge_index, batch):
    """Pure-numpy general path (only used if the expected input structure is
    absent; inputs from setup_inputs always take the device path)."""
    f32 = np.float32
    E = members.shape[0]
    num_entries = members.size
    x = np.clip(pos_set @ W_enc + b_enc, -1.0, 1.0).astype(f32)
    xe = x[members.reshape(-1)]
    cnt = np.zeros(E, f32)
    np.add.at(cnt, batch, 1.0)
    mean = np.zeros((E, x.shape[1]), f32)
    np.add.at(mean, batch, xe)
    mean /= cnt[:, None]
    ctr = xe - gn_alpha * mean[batch]
    var = np.zeros((E, x.shape[1]), f32)
    np.add.at(var, batch, ctr * ctr)
    var /= cnt[:, None]
    xe = gn_gamma * ctr / np.sqrt(var + EPS)[batch] + gn_beta
    src, dst = edge_index[0], edge_index[1]
    deg = np.zeros(num_entries, f32)
    np.add.at(deg, dst, 1.0)
    w = -1.0 / np.sqrt(deg[src] * deg[dst])

    def lhat(h):
        out = np.zeros_like(h)
        np.add.at(out, dst, w[:, None] * h[src])
        return out

    tx0 = xe
    tx1 = lhat(tx0)
    out = tx0 @ cheb_W[0] + tx1 @ cheb_W[1]
    tkm1, tkm2 = tx1, tx0
    for k in range(2, cheb_W.shape[0]):
        tk = 2.0 * lhat(tkm1) - tkm2
        out = out + tk @ cheb_W[k]
        tkm1, tkm2 = tk, tkm1
    h = np.clip(out + cheb_b, -1.0, 1.0)
    ymax = np.full((E, h.shape[1]), -np.inf, f32)
    ymin = np.full((E, h.shape[1]), np.inf, f32)
    np.maximum.at(ymax, batch, h)
    np.minimum.at(ymin, batch, h)
    ynorm = np.zeros((E, h.shape[1]), f32)
    np.add.at(ynorm, batch, h * h)
    ynorm = np.sqrt(ynorm / cnt[:, None])
    y = np.concatenate([ymax - ymin, ynorm], axis=1)
    logits = y @ lin_W + lin_b
    return (1.0 / (1.0 + np.exp(-logits))).squeeze(-1).astype(f32)


def _has_window_structure(members, edge_index, batch):
    E, Mm = members.shape
    if Mm != M:
        return False
    base = members[:, 0]
    if not (members == base[:, None] + np.arange(M, dtype=members.dtype)).all():
        return False
    if not (batch == np.repeat(np.arange(E, dtype=batch.dtype), M)).all():
        return False
    row, col = np.where(~np.eye(M, dtype=bool))
    offs = np.arange(E, dtype=np.int64)[:, None] * M
    ei = np.stack([(offs + row[None, :]).ravel(), (offs + col[None, :]).ravel()])
    return (edge_index == ei).all()


def kernel(pos_set, W_enc, b_enc, gn_gamma, gn_beta, gn_alpha, cheb_W, cheb_b,
           lin_W, lin_b, members, edge_index, batch):
    pos_set = np.asarray(pos_set, np.float32)
    members = np.asarray(members)
    edge_index = np.asarray(edge_index)
    batch = np.asarray(batch)
    if not _has_window_structure(members, edge_index, batch):
        return _np_fallback(
            pos_set, np.asarray(W_enc, np.float32),
            np.asarray(b_enc, np.float32), np.asarray(gn_gamma, np.float32),
            np.asarray(gn_beta, np.float32), np.asarray(gn_alpha, np.float32),
            np.asarray(cheb_W, np.float32), np.asarray(cheb_b, np.float32),
            np.asarray(lin_W, np.float32), np.asarray(lin_b, np.float32),
            members, edge_index, batch)

    N = pos_set.shape[0]
    base_all = members[:, 0].astype(np.int64)
    # duplicate windows (same base) give identical outputs: compute unique
    # windows only, scatter back through the inverse map at the end
    uniq, inv = np.unique(base_all, return_inverse=True)
    EU = uniq.size
    base = uniq                                              # sorted
    node_span = (N + N_CORES - 1) // N_CORES
    u_pad = ((node_span + M + 511) // 512 + 1) * 512
    # quantile split: equal window count per core (bases already sorted)
    ecnt = (EU + N_CORES - 1) // N_CORES
    counts = np.array([max(0, min(ecnt, EU - c * ecnt)) for c in range(N_CORES)])
    offs_pre = np.concatenate([[0], np.cumsum(counts)])
    node_lo = np.zeros(N_CORES, np.int64)
    ok = True
    for c in range(N_CORES):
        lo_i, hi_i = offs_pre[c], offs_pre[c + 1]
        if hi_i == lo_i:
            node_lo[c] = 0
            continue
        node_lo[c] = base[lo_i]
        if base[hi_i - 1] + M - node_lo[c] > u_pad:
            ok = False
            break
    if not ok:
        core_of = np.minimum(base // node_span, N_CORES - 1)
        counts = np.bincount(core_of, minlength=N_CORES)
        offs_pre = np.concatenate([[0], np.cumsum(counts)])
        node_lo = np.arange(N_CORES, dtype=np.int64) * node_span
    cmax = int(counts.max())
    # lead with small chunks so the first gathers (and their stats) are
    # ready within a few encoder batches and fill the otherwise-idle DVE
    chunks = []
    left = cmax
    while left > 0:
        take = min(GSZ, ((left + 127) // 128) * 128)
        chunks.append(take)
        left -= take
    chunks = tuple(chunks)
    e_pad = int(sum(chunks))
    coffs = np.concatenate([[0], np.cumsum(chunks)]).astype(np.int64)

    nb_ = u_pad // 512
    # per-chunk: how many 512-node x-write batches the gather depends on
    # (max over cores, from the actual window bases)
    gdeps = []
    for c in range(len(chunks)):
        mx = 0
        for cc in range(N_CORES):
            lo_i = offs_pre[cc] + coffs[c]
            hi_i = min(offs_pre[cc] + coffs[c + 1], offs_pre[cc + 1])
            if hi_i > lo_i:
                mx = max(mx, int((base[lo_i:hi_i] - node_lo[cc]).max()))
        gdeps.append(min(nb_, max(1, (mx + M + 511) // 512)))
    gdeps = tuple(gdeps)
    benc_zero = bool(np.all(np.asarray(b_enc) == 0.0))
    key = (u_pad, chunks, gdeps, benc_zero)
    if key not in _GRAPH_CACHE:
        _GRAPH_CACHE[key] = _build_graph(u_pad, chunks, gdeps, benc_zero)
    nc = _GRAPH_CACHE[key]

    shared = _fold_weights(W_enc, b_enc, gn_gamma, gn_beta, gn_alpha, cheb_W,
                           cheb_b, lin_W, lin_b)
    nb = u_pad // 512
    ns16 = e_pad // 16

    in_maps = []
    for c in range(N_CORES):
        lo = int(node_lo[c])
        sl = pos_set[lo:min(lo + u_pad, N)]
        if sl.shape[0] < u_pad:
            sl = np.concatenate(
                [sl, np.zeros((u_pad - sl.shape[0], F), np.float32)], 0)
        # posT[b, p, k, u'] = sl[512b+u', 128k+p]
        posT = np.ascontiguousarray(
            sl.reshape(nb, 512, 4, 128).transpose(0, 3, 2, 1)
            .astype(np.float16))
        wins = base[offs_pre[c]:offs_pre[c + 1]]
        loc = (wins - lo).astype(np.int64)
        idx = np.zeros(e_pad, np.int16)
        # place each chunk's windows at its padded chunk offset
        for ci in range(len(chunks)):
            seg = loc[coffs[ci]:min(coffs[ci + 1], loc.size)]
            idx[int(coffs[ci]):int(coffs[ci]) + seg.size] = \
                seg.astype(np.int16)
        # wrapped layout: element i lives at [i % 16, i // 16], replicated
        # across the eight 16-partition groups
        w16 = idx.reshape(ns16, 16).T           # [16, ns16]
        m = dict(shared)
        m["posT"] = posT
        m["idx"] = np.ascontiguousarray(np.tile(w16, (8, 1)))
        in_maps.append(m)

    from concourse.bass_utils import run_bass_kernel_spmd

    trace = bool(os.environ.get("CHESHIRE_TRACE"))
    res = run_bass_kernel_spmd(nc, in_maps, core_ids=list(range(N_CORES)),
                               trace=trace)
    global LAST_EXEC_NS, LAST_RESULT
    LAST_EXEC_NS = res.exec_time_ns
    LAST_RESULT = res
    out_u = np.zeros(EU, np.float32)
    for c in range(N_CORES):
        cnt_c = int(counts[c])
        if cnt_c == 0:
            continue
        vals = np.asarray(res.results[c]["out"], np.float32).reshape(-1)
        # undo chunk padding: chunk ci's real windows sit at coffs[ci]
        got = np.empty(cnt_c, np.float32)
        for ci in range(len(chunks)):
            s = int(coffs[ci])
            e = min(int(coffs[ci + 1]), cnt_c)
            if e > s:
                got[s:e] = vals[s:e]
        out_u[offs_pre[c]:offs_pre[c] + cnt_c] = got
    return out_u[inv]
